# revision 22
# baseline (speedup 1.0000x reference)
"""Chamfer-distance loss (nn_CDLoss) on 8 Trainium2 NeuronCores.

Strategy (data parallel over graphs, 2 graphs per core):

  Distances via one K=13 bf16 matmul (hi/lo split keeps fp32-grade accuracy):
      p = ph + pl (bf16 hi/lo), n_p = ||p||^2 = nh_p + nl_p (bf16 hi/lo)
      row-enc p : ( ph[3], pl[3], ph[3], nh_p, nl_p, 1, 1 )
      col-enc q : (-2qh[3], -2qh[3], -2ql[3], w, w, nh_q, nl_q )
      (row.T @ col)[p,q] = -2(ph qh + pl qh + ph ql) + w*n_p + n_q ~= ||p-q||^2

  Column-pair trick: d is LINEAR in the column encoding vector, so for column
  pairs (m, m+H) the host emits SUM and DIFF encodings. The PE then computes
  S = d_A + d_B and D = d_A - d_B with the SAME matmul count, the otherwise
  idle scalar engine computes |D|, and one fused vector-engine
  tensor_tensor_reduce produces min(d_A, d_B) = (S - |D|)/2 AND its row-min —
  so the vector engine (the bottleneck) touches each distance pair once
  instead of each distance.

  Padding: fake rows are all-zero (row-min contributes 0); to_dense_batch's
  zero pads are represented by one zero-point column; alignment pad columns
  replicate a real candidate (never BIG, which would destroy fp32 precision
  in the S/|D| cancellation).

  Per (graph, direction): tile rows by 128 (PE row groups alternate q0/q32 so
  weight loads pull ahead), pair-columns chunked through PSUM, row-min
  partials reduced on the vector engine, per-lane sums DMA'd out.
  Host sums the 8 cores' [128, 2*GPC] partials and divides by G*n_max.
"""

import math

import ml_dtypes
import numpy as np

BF16 = ml_dtypes.bfloat16
K = 13
N_CORES = 8
FUSED_TTR = False  # fused (S-|D|)*0.5 + row-min in one DVE instruction


# --------------------------------------------------------------------------
# Device kernel
# --------------------------------------------------------------------------

def build_nc(P: int, gpc: int):
    """Build + compile the per-core Bass/Tile kernel.

    P   : padded points per cloud (multiple of 128); H = P//2 column pairs
    gpc : graphs per core
    Inputs  rowx, rowy : [gpc, K, P] bf16 ; colS*, colD* : [gpc, K, H] bf16
    Output  out : [128, 2*gpc] f32 — per-lane sums of row-mins, one column
            per (graph, direction).
    """
    import concourse.bass as bass
    import concourse.mybir as mybir
    from concourse import bacc, tile

    f32 = mybir.dt.float32
    bf16 = mybir.dt.bfloat16
    T = P // 128
    H = P // 2
    # PSUM chunking of the pair-column axis: 1024 f32 (2 banks) per chunk for
    # each of S and D, x2 pool slots = 8 banks. Tail chunk first so the
    # post-row-tile-boundary catch-up reduce is the small one.
    CH = 1024
    chunks = []
    c0 = 0
    while c0 < H:
        w = min(CH, H - c0)
        chunks.append((c0, w))
        c0 += w
    chunks.sort(key=lambda cw: cw[1])
    n_ch = len(chunks)

    nc = bacc.Bacc("TRN2", target_bir_lowering=False, debug=False)

    rowx = nc.dram_tensor("rowx", [gpc, K, P], bf16, kind="ExternalInput")
    rowy = nc.dram_tensor("rowy", [gpc, K, P], bf16, kind="ExternalInput")
    colSx = nc.dram_tensor("colSx", [gpc, K, H], bf16, kind="ExternalInput")
    colDx = nc.dram_tensor("colDx", [gpc, K, H], bf16, kind="ExternalInput")
    colSy = nc.dram_tensor("colSy", [gpc, K, H], bf16, kind="ExternalInput")
    colDy = nc.dram_tensor("colDy", [gpc, K, H], bf16, kind="ExternalInput")
    out = nc.dram_tensor("out", [128, 2 * gpc], f32, kind="ExternalOutput")

    with tile.TileContext(nc) as tc:
        with (
            tc.tile_pool(name="enc", bufs=2) as enc_pool,
            tc.tile_pool(name="work", bufs=3) as work_pool,
            tc.tile_pool(name="mins", bufs=2) as min_pool,
            tc.tile_pool(name="res", bufs=1) as res_pool,
            tc.tile_pool(name="ps", bufs=2, space="PSUM") as ps_pool,
        ):
            out_sb = res_pool.tile([128, 2 * gpc], f32, name="out_sb")

            pairs = []
            for g in range(gpc):
                pairs.append((rowx[g], colSy[g], colDy[g]))  # cham_x
                pairs.append((rowy[g], colSx[g], colDx[g]))  # cham_y

            for pi, (row_dram, colS_dram, colD_dram) in enumerate(pairs):
                # Encodings replicated at partition offsets 0 and 32 so
                # consecutive row tiles use different PE row groups (q0/q32):
                # the next tile's LDWEIGHTS then pulls ahead of the in-flight
                # matmuls instead of waiting for the array to drain.
                row_sb = enc_pool.tile([32 + K, P], bf16, name="row_sb", tag="row")
                colS_sb = enc_pool.tile([32 + K, H], bf16, name="colS_sb", tag="cs")
                colD_sb = enc_pool.tile([32 + K, H], bf16, name="colD_sb", tag="cd")
                for dst, src in ((row_sb, row_dram), (colS_sb, colS_dram),
                                 (colD_sb, colD_dram)):
                    nc.sync.dma_start(dst[0:K, :], src)
                    nc.sync.dma_start(dst[32:32 + K, :], src)

                # pm[:, i*n_ch + ci] = row-min over pair-chunk ci of row tile i
                pm = min_pool.tile([128, T * n_ch], f32, name="pm", tag="pm")
                rowmins = min_pool.tile([128, T], f32, name="rowmins", tag="rm")
                for i in range(T):
                    q = 32 * (i % 2)
                    lhsT = row_sb[q:q + K, i * 128:(i + 1) * 128]
                    for ci, (cstart, w) in enumerate(chunks):
                        psS = ps_pool.tile([128, w], f32, name="psS", tag="psS")
                        psD = ps_pool.tile([128, w], f32, name="psD", tag="psD")
                        for ps, csb in ((psS, colS_sb), (psD, colD_sb)):
                            for j in range(0, w, 512):
                                n = min(512, w - j)
                                nc.tensor.matmul(
                                    ps[:, j:j + n],
                                    lhsT,
                                    csb[q:q + K, cstart + j:cstart + j + n],
                                )
                        absD = work_pool.tile([128, w], f32, name="absD", tag="absd")
                        nc.scalar.activation(
                            absD[:, :w], psD[:, :w],
                            mybir.ActivationFunctionType.Abs,
                        )
                        scr = work_pool.tile([128, w], f32, name="scr", tag="scr")
                        if FUSED_TTR:
                            nc.vector.tensor_tensor_reduce(
                                out=scr[:, :w],
                                in0=psS[:, :w],
                                in1=absD[:, :w],
                                scale=0.5,
                                scalar=1e30,
                                op0=mybir.AluOpType.subtract,
                                op1=mybir.AluOpType.min,
                                accum_out=pm[:, i * n_ch + ci:i * n_ch + ci + 1],
                            )
                        else:
                            # unfused fallback: (S - |D|) then row-min; the
                            # missing *0.5 is applied on the host
                            nc.vector.tensor_tensor(
                                scr[:, :w], psS[:, :w], absD[:, :w],
                                op=mybir.AluOpType.subtract,
                            )
                            nc.vector.tensor_reduce(
                                pm[:, i * n_ch + ci:i * n_ch + ci + 1],
                                scr[:, :w],
                                axis=mybir.AxisListType.X,
                                op=mybir.AluOpType.min,
                            )
                nc.vector.tensor_reduce(
                    rowmins[:], pm[:].rearrange("p (t c) -> p t c", c=n_ch),
                    axis=mybir.AxisListType.X, op=mybir.AluOpType.min,
                )
                nc.vector.reduce_sum(
                    out_sb[:, pi:pi + 1], rowmins[:], axis=mybir.AxisListType.X,
                )

            nc.sync.dma_start(out[:], out_sb[:])

    nc.compile()
    return nc


# --------------------------------------------------------------------------
# Host-side encode / shard / gather
# --------------------------------------------------------------------------

def _encode_rows(v: np.ndarray, c: int, P: int):
    """Row encoding [K,P] bf16 of the c real points in v (fake rows zero)."""
    row = np.zeros((K, P), np.float32)
    if c:
        v = v.astype(np.float32)
        vh = v.astype(BF16).astype(np.float32)
        vl = (v - vh).astype(BF16).astype(np.float32)
        n = (v.astype(np.float64) ** 2).sum(1)
        nh = n.astype(BF16).astype(np.float64)
        nl = (n - nh).astype(BF16).astype(np.float32)
        row[0:3, :c] = vh.T
        row[3:6, :c] = vl.T
        row[6:9, :c] = vh.T
        row[9, :c] = nh
        row[10, :c] = nl
        row[11, :c] = 1.0
        row[12, :c] = 1.0
    return row.astype(BF16)


def _virtual_cols(v: np.ndarray, c: int, P: int, n_max: int):
    """Candidate columns as fp32/fp64 (q [P,3], w [P], n [P]).

    Real points, then (if the cloud is shorter than n_max) one zero-point
    column standing in for all to_dense_batch zero pads; alignment padding
    replicates an existing candidate so no sentinel values are needed.
    """
    q = np.zeros((P, 3), np.float32)
    w = np.ones(P, np.float32)
    n = np.zeros(P, np.float64)
    if c:
        v = v.astype(np.float32)
        q[:c] = v
        n[:c] = (v.astype(np.float64) ** 2).sum(1)
    if c and c >= n_max:
        # fullest graph: zero is NOT a candidate; pads replicate point 0
        q[c:] = v[0]
        n[c:] = n[0]
    # else: columns c.. stay the zero-point candidate (w=1, n=0)
    return q, w, n


def _encode_cols(q, w, n):
    """13-row bf16 column encoding from fp32/fp64 virtual columns."""
    Pn = q.shape[0]
    col = np.zeros((K, Pn), np.float32)
    m = (-2.0 * q).astype(np.float32)
    a = m.astype(BF16).astype(np.float32)
    b = (m - a).astype(BF16).astype(np.float32)
    nh = n.astype(BF16).astype(np.float64)
    nl = (n - nh).astype(BF16).astype(np.float32)
    col[0:3] = a.T
    col[3:6] = a.T
    col[6:9] = b.T
    col[9] = w
    col[10] = w
    col[11] = nh
    col[12] = nl
    return col.astype(BF16)


def _encode_col_pair(v: np.ndarray, c: int, P: int, n_max: int):
    """SUM and DIFF column-pair encodings ([K,H] bf16 each), H = P//2."""
    q, w, n = _virtual_cols(v, c, P, n_max)
    H = P // 2
    colS = _encode_cols(q[:H] + q[H:], w[:H] + w[H:], n[:H] + n[H:])
    colD = _encode_cols(q[:H] - q[H:], w[:H] - w[H:], n[:H] - n[H:])
    return colS, colD


def prepare(pred, target, batch):
    """Returns (in_maps, num_graphs, n_max, P, gpc)."""
    pred = np.ascontiguousarray(np.asarray(pred), dtype=np.float32)
    target = np.ascontiguousarray(np.asarray(target), dtype=np.float32)
    batch = np.asarray(batch).astype(np.int64)

    num_graphs = int(batch.max()) + 1
    counts = np.bincount(batch, minlength=num_graphs)
    n_max = int(counts.max())
    P = ((n_max + 127) // 128) * 128
    H = P // 2
    gpc = max(1, math.ceil(num_graphs / N_CORES))
    starts = np.zeros(num_graphs + 1, np.int64)
    np.cumsum(counts, out=starts[1:])

    empty = np.zeros((0, 3), np.float32)
    in_maps = []
    for core in range(N_CORES):
        m = {"rowx": np.zeros((gpc, K, P), BF16),
             "rowy": np.zeros((gpc, K, P), BF16),
             "colSx": np.zeros((gpc, K, H), BF16),
             "colDx": np.zeros((gpc, K, H), BF16),
             "colSy": np.zeros((gpc, K, H), BF16),
             "colDy": np.zeros((gpc, K, H), BF16)}
        for slot in range(gpc):
            g = core * gpc + slot
            if g < num_graphs:
                c = int(counts[g])
                x = pred[starts[g]:starts[g + 1]]
                y = target[starts[g]:starts[g + 1]]
            else:
                c, x, y = 0, empty, empty  # unused slot contributes 0
            m["rowx"][slot] = _encode_rows(x, c, P)
            m["rowy"][slot] = _encode_rows(y, c, P)
            m["colSx"][slot], m["colDx"][slot] = _encode_col_pair(x, c, P, n_max)
            m["colSy"][slot], m["colDy"][slot] = _encode_col_pair(y, c, P, n_max)
        in_maps.append(m)
    return in_maps, num_graphs, n_max, P, gpc


def run(pred, target, batch, trace=False, **spmd_kwargs):
    """Full pipeline. Returns (loss_scalar, BassKernelResults)."""
    from concourse.bass_utils import run_bass_kernel_spmd

    in_maps, num_graphs, n_max, P, gpc = prepare(pred, target, batch)
    nc = build_nc(P, gpc)
    res = run_bass_kernel_spmd(
        nc, in_maps, core_ids=list(range(N_CORES)), trace=trace, **spmd_kwargs,
    )
    total = 0.0
    for core in range(N_CORES):
        total += res.results[core]["out"].astype(np.float64).sum()
    if not FUSED_TTR:
        total *= 0.5  # unfused path skips the on-device *0.5
    loss = np.float32(total / (num_graphs * n_max))
    return loss, res


def kernel(pred, target, batch):
    loss, _ = run(pred, target, batch, trace=False)
    return loss


# revision 23
# speedup vs baseline: 1.0126x; 1.0126x over previous
"""Chamfer-distance loss (nn_CDLoss) on 8 Trainium2 NeuronCores.

Strategy (data parallel over graphs, 2 graphs per core):

  Distances via one K=13 bf16 matmul (hi/lo split keeps fp32-grade accuracy;
  fp32 matmuls on TRN2 run in slow LOW_HIGH mode so we do the split ourselves,
  dropping only the lo*lo cross term, ~1e-6 relative on the loss):
      p = ph + pl (bf16 hi/lo), n_p = ||p||^2 = nh_p + nl_p (bf16 hi/lo)
      row-enc p : ( ph[3], pl[3], ph[3], nh_p, nl_p, 1, 1 )
      col-enc q : (-2qh[3], -2qh[3], -2ql[3], w, w, nh_q, nl_q )
      (row.T @ col)[p,q] = -2(ph qh + pl qh + ph ql) + w*n_p + n_q ~= ||p-q||^2

  Padding: fake rows are all-zero (their row-min is 0, adds nothing to the
  sum); to_dense_batch's zero pads are represented by one zero-point column;
  alignment pad columns replicate an existing candidate.

  Per (graph, direction): tile rows by 128 (PE row groups alternate q0/q32 so
  the next tile's weight load pulls ahead of in-flight matmuls), columns
  chunked 1024 wide through PSUM with 4 pool slots so the PE runs far enough
  ahead to keep the vector engine (the bottleneck) 100% busy on row-min
  reduces. Row-min partials land in pm, reduced to per-row mins, then per-lane
  sums are DMA'd out. Host sums the 8 cores' [128, 2*GPC] partials and
  divides by G*n_max.
"""

import math

import ml_dtypes
import numpy as np

BF16 = ml_dtypes.bfloat16
K = 13
N_CORES = 8


# --------------------------------------------------------------------------
# Device kernel
# --------------------------------------------------------------------------

def build_nc(P: int, gpc: int):
    """Build + compile the per-core Bass/Tile kernel.

    P   : padded points per cloud (multiple of 128)
    gpc : graphs per core
    Inputs  rowx, colx, rowy, coly : [gpc, K, P] bf16
    Output  out : [128, 2*gpc] f32 — per-lane sums of row-mins, one column
            per (graph, direction).
    """
    import concourse.bass as bass
    import concourse.mybir as mybir
    from concourse import bacc, tile

    f32 = mybir.dt.float32
    bf16 = mybir.dt.bfloat16
    T = P // 128
    # PSUM chunking of the column axis: 1024 f32 (2 banks) per chunk, 4 pool
    # slots = 8 banks. Tail chunk first so the post-row-tile-boundary
    # catch-up reduce is the small one.
    CH = 1024
    chunks = []
    c0 = 0
    while c0 < P:
        w = min(CH, P - c0)
        chunks.append((c0, w))
        c0 += w
    chunks.sort(key=lambda cw: cw[1])
    n_ch = len(chunks)

    nc = bacc.Bacc("TRN2", target_bir_lowering=False, debug=False)

    rowx = nc.dram_tensor("rowx", [gpc, K, P], bf16, kind="ExternalInput")
    colx = nc.dram_tensor("colx", [gpc, K, P], bf16, kind="ExternalInput")
    rowy = nc.dram_tensor("rowy", [gpc, K, P], bf16, kind="ExternalInput")
    coly = nc.dram_tensor("coly", [gpc, K, P], bf16, kind="ExternalInput")
    out = nc.dram_tensor("out", [128, 2 * gpc], f32, kind="ExternalOutput")

    with tile.TileContext(nc) as tc:
        with (
            tc.tile_pool(name="enc", bufs=2) as enc_pool,
            tc.tile_pool(name="mins", bufs=2) as min_pool,
            tc.tile_pool(name="res", bufs=1) as res_pool,
            tc.tile_pool(name="ps", bufs=4, space="PSUM") as ps_pool,
        ):
            out_sb = res_pool.tile([128, 2 * gpc], f32, name="out_sb")

            pairs = []
            for g in range(gpc):
                pairs.append((rowx[g], coly[g]))  # cham_x direction
                pairs.append((rowy[g], colx[g]))  # cham_y direction

            for pi, (row_dram, col_dram) in enumerate(pairs):
                # Encodings replicated at partition offsets 0 and 32 so
                # consecutive row tiles use different PE row groups (q0/q32):
                # the next tile's LDWEIGHTS then pulls ahead of the in-flight
                # matmuls instead of waiting for the array to drain.
                row_sb = enc_pool.tile([32 + K, P], bf16, name="row_sb", tag="row")
                col_sb = enc_pool.tile([32 + K, P], bf16, name="col_sb", tag="col")
                nc.sync.dma_start(row_sb[0:K, :], row_dram)
                nc.sync.dma_start(row_sb[32:32 + K, :], row_dram)
                nc.sync.dma_start(col_sb[0:K, :], col_dram)
                nc.sync.dma_start(col_sb[32:32 + K, :], col_dram)

                # pm[:, i*n_ch + ci] = row-min over chunk ci of row tile i
                pm = min_pool.tile([128, T * n_ch], f32, name="pm", tag="pm")
                rowmins = min_pool.tile([128, T], f32, name="rowmins", tag="rm")
                for i in range(T):
                    q = 32 * (i % 2)
                    lhsT = row_sb[q:q + K, i * 128:(i + 1) * 128]
                    for ci, (cstart, w) in enumerate(chunks):
                        ps = ps_pool.tile([128, w], f32, name="ps", tag="ps")
                        for j in range(0, w, 512):
                            n = min(512, w - j)
                            nc.tensor.matmul(
                                ps[:, j:j + n],
                                lhsT,
                                col_sb[q:q + K, cstart + j:cstart + j + n],
                            )
                        nc.vector.tensor_reduce(
                            pm[:, i * n_ch + ci:i * n_ch + ci + 1], ps[:, :w],
                            axis=mybir.AxisListType.X, op=mybir.AluOpType.min,
                        )
                nc.vector.tensor_reduce(
                    rowmins[:], pm[:].rearrange("p (t c) -> p t c", c=n_ch),
                    axis=mybir.AxisListType.X, op=mybir.AluOpType.min,
                )
                nc.vector.reduce_sum(
                    out_sb[:, pi:pi + 1], rowmins[:], axis=mybir.AxisListType.X,
                )

            nc.sync.dma_start(out[:], out_sb[:])

    nc.compile()
    return nc


# --------------------------------------------------------------------------
# Host-side encode / shard / gather
# --------------------------------------------------------------------------

def _encode_rows(v: np.ndarray, c: int, P: int):
    """Row encoding [K,P] bf16 of the c real points in v (fake rows zero)."""
    row = np.zeros((K, P), np.float32)
    if c:
        v = v.astype(np.float32)
        vh = v.astype(BF16).astype(np.float32)
        vl = (v - vh).astype(BF16).astype(np.float32)
        n = (v.astype(np.float64) ** 2).sum(1)
        nh = n.astype(BF16).astype(np.float64)
        nl = (n - nh).astype(BF16).astype(np.float32)
        row[0:3, :c] = vh.T
        row[3:6, :c] = vl.T
        row[6:9, :c] = vh.T
        row[9, :c] = nh
        row[10, :c] = nl
        row[11, :c] = 1.0
        row[12, :c] = 1.0
    return row.astype(BF16)


def _encode_cols(v: np.ndarray, c: int, P: int, n_max: int):
    """Column encoding [K,P] bf16: c real candidate points, then (if the
    cloud is shorter than n_max) a zero-point candidate standing in for all
    to_dense_batch zero pads; alignment padding replicates a candidate."""
    q = np.zeros((P, 3), np.float32)
    w = np.ones(P, np.float32)
    n = np.zeros(P, np.float64)
    if c:
        v = v.astype(np.float32)
        q[:c] = v
        n[:c] = (v.astype(np.float64) ** 2).sum(1)
    if c and c >= n_max:
        # fullest graph: zero is NOT a candidate; pads replicate point 0
        q[c:] = v[0]
        n[c:] = n[0]
    # else: columns c.. stay the zero-point candidate (w=1, n=0)

    col = np.zeros((K, P), np.float32)
    m = (-2.0 * q).astype(np.float32)
    a = m.astype(BF16).astype(np.float32)
    b = (m - a).astype(BF16).astype(np.float32)
    nh = n.astype(BF16).astype(np.float64)
    nl = (n - nh).astype(BF16).astype(np.float32)
    col[0:3] = a.T
    col[3:6] = a.T
    col[6:9] = b.T
    col[9] = w
    col[10] = w
    col[11] = nh
    col[12] = nl
    return col.astype(BF16)


def prepare(pred, target, batch):
    """Returns (in_maps, num_graphs, n_max, P, gpc)."""
    pred = np.ascontiguousarray(np.asarray(pred), dtype=np.float32)
    target = np.ascontiguousarray(np.asarray(target), dtype=np.float32)
    batch = np.asarray(batch).astype(np.int64)

    num_graphs = int(batch.max()) + 1
    counts = np.bincount(batch, minlength=num_graphs)
    n_max = int(counts.max())
    P = ((n_max + 127) // 128) * 128
    gpc = max(1, math.ceil(num_graphs / N_CORES))
    starts = np.zeros(num_graphs + 1, np.int64)
    np.cumsum(counts, out=starts[1:])

    empty = np.zeros((0, 3), np.float32)
    in_maps = []
    for core in range(N_CORES):
        m = {k: np.zeros((gpc, K, P), BF16)
             for k in ("rowx", "colx", "rowy", "coly")}
        for slot in range(gpc):
            g = core * gpc + slot
            if g < num_graphs:
                c = int(counts[g])
                x = pred[starts[g]:starts[g + 1]]
                y = target[starts[g]:starts[g + 1]]
            else:
                c, x, y = 0, empty, empty  # unused slot contributes 0
            m["rowx"][slot] = _encode_rows(x, c, P)
            m["colx"][slot] = _encode_cols(x, c, P, n_max)
            m["rowy"][slot] = _encode_rows(y, c, P)
            m["coly"][slot] = _encode_cols(y, c, P, n_max)
        in_maps.append(m)
    return in_maps, num_graphs, n_max, P, gpc


def run(pred, target, batch, trace=False, **spmd_kwargs):
    """Full pipeline. Returns (loss_scalar, BassKernelResults)."""
    from concourse.bass_utils import run_bass_kernel_spmd

    in_maps, num_graphs, n_max, P, gpc = prepare(pred, target, batch)
    nc = build_nc(P, gpc)
    res = run_bass_kernel_spmd(
        nc, in_maps, core_ids=list(range(N_CORES)), trace=trace, **spmd_kwargs,
    )
    total = 0.0
    for core in range(N_CORES):
        total += res.results[core]["out"].astype(np.float64).sum()
    loss = np.float32(total / (num_graphs * n_max))
    return loss, res


def kernel(pred, target, batch):
    loss, _ = run(pred, target, batch, trace=False)
    return loss


# revision 24
# speedup vs baseline: 1.0129x; 1.0003x over previous
"""Chamfer-distance loss (nn_CDLoss) on 8 Trainium2 NeuronCores.

Strategy (data parallel over graphs, 2 graphs per core):

  Distances via one K=13 bf16 matmul (hi/lo split keeps fp32-grade accuracy;
  fp32 matmuls on TRN2 run in slow LOW_HIGH mode so we do the split ourselves,
  dropping only the lo*lo cross term, ~1e-6 relative on the loss):
      p = ph + pl (bf16 hi/lo), n_p = ||p||^2 = nh_p + nl_p (bf16 hi/lo)
      row-enc p : ( ph[3], pl[3], ph[3], nh_p, nl_p, 1, 1 )
      col-enc q : (-2qh[3], -2qh[3], -2ql[3], w, w, nh_q, nl_q )
      (row.T @ col)[p,q] = -2(ph qh + pl qh + ph ql) + w*n_p + n_q ~= ||p-q||^2

  Padding: fake rows are all-zero (their row-min is 0, adds nothing to the
  sum); to_dense_batch's zero pads are represented by one zero-point column;
  alignment pad columns replicate an existing candidate.

  Per (graph, direction): tile rows by 128 (PE row groups alternate q0/q32 so
  the next tile's weight load pulls ahead of in-flight matmuls), columns
  chunked 1024 wide through PSUM with 4 pool slots so the PE runs far enough
  ahead to keep the vector engine (the bottleneck) 100% busy on row-min
  reduces. Row-min partials land in pm, reduced to per-row mins, then per-lane
  sums are DMA'd out. Host sums the 8 cores' [128, 2*GPC] partials and
  divides by G*n_max.
"""

import math
import os
import sys

# concourse normally comes from PYTHONPATH (/root/.axon_site/_ro/trn_rl_repo);
# fall back to the /opt copy if the env var is missing.
for _p in ("/opt/trn_rl_repo", "/root/.axon_site/_ro/trn_rl_repo"):
    if os.path.isdir(_p) and _p not in sys.path:
        sys.path.append(_p)

import ml_dtypes
import numpy as np

BF16 = ml_dtypes.bfloat16
K = 13
N_CORES = 8


# --------------------------------------------------------------------------
# Device kernel
# --------------------------------------------------------------------------

def build_nc(P: int, gpc: int):
    """Build + compile the per-core Bass/Tile kernel.

    P   : padded points per cloud (multiple of 128)
    gpc : graphs per core
    Inputs  rowx, colx, rowy, coly : [gpc, K, P] bf16
    Output  out : [128, 2*gpc] f32 — per-lane sums of row-mins, one column
            per (graph, direction).
    """
    import concourse.bass as bass
    import concourse.mybir as mybir
    from concourse import bacc, tile

    f32 = mybir.dt.float32
    bf16 = mybir.dt.bfloat16
    T = P // 128
    # PSUM chunking of the column axis: 1024 f32 (2 banks) per chunk, 4 pool
    # slots = 8 banks. Tail chunk first so the post-row-tile-boundary
    # catch-up reduce is the small one.
    CH = 1024
    chunks = []
    c0 = 0
    while c0 < P:
        w = min(CH, P - c0)
        chunks.append((c0, w))
        c0 += w
    chunks.sort(key=lambda cw: cw[1])
    n_ch = len(chunks)

    nc = bacc.Bacc("TRN2", target_bir_lowering=False, debug=False)

    rowx = nc.dram_tensor("rowx", [gpc, K, P], bf16, kind="ExternalInput")
    colx = nc.dram_tensor("colx", [gpc, K, P], bf16, kind="ExternalInput")
    rowy = nc.dram_tensor("rowy", [gpc, K, P], bf16, kind="ExternalInput")
    coly = nc.dram_tensor("coly", [gpc, K, P], bf16, kind="ExternalInput")
    out = nc.dram_tensor("out", [128, 2 * gpc], f32, kind="ExternalOutput")

    with tile.TileContext(nc) as tc:
        with (
            tc.tile_pool(name="enc", bufs=2) as enc_pool,
            tc.tile_pool(name="mins", bufs=2) as min_pool,
            tc.tile_pool(name="res", bufs=1) as res_pool,
            tc.tile_pool(name="ps", bufs=4, space="PSUM") as ps_pool,
        ):
            out_sb = res_pool.tile([128, 2 * gpc], f32, name="out_sb")

            pairs = []
            for g in range(gpc):
                pairs.append((rowx[g], coly[g]))  # cham_x direction
                pairs.append((rowy[g], colx[g]))  # cham_y direction

            for pi, (row_dram, col_dram) in enumerate(pairs):
                # Encodings replicated at partition offsets 0 and 32 so
                # consecutive row tiles use different PE row groups (q0/q32):
                # the next tile's LDWEIGHTS then pulls ahead of the in-flight
                # matmuls instead of waiting for the array to drain.
                row_sb = enc_pool.tile([32 + K, P], bf16, name="row_sb", tag="row")
                col_sb = enc_pool.tile([32 + K, P], bf16, name="col_sb", tag="col")
                nc.sync.dma_start(row_sb[0:K, :], row_dram)
                nc.sync.dma_start(row_sb[32:32 + K, :], row_dram)
                nc.sync.dma_start(col_sb[0:K, :], col_dram)
                nc.sync.dma_start(col_sb[32:32 + K, :], col_dram)

                # pm[:, i*n_ch + ci] = row-min over chunk ci of row tile i
                pm = min_pool.tile([128, T * n_ch], f32, name="pm", tag="pm")
                rowmins = min_pool.tile([128, T], f32, name="rowmins", tag="rm")
                for i in range(T):
                    q = 32 * (i % 2)
                    lhsT = row_sb[q:q + K, i * 128:(i + 1) * 128]
                    for ci, (cstart, w) in enumerate(chunks):
                        ps = ps_pool.tile([128, w], f32, name="ps", tag="ps")
                        for j in range(0, w, 512):
                            n = min(512, w - j)
                            nc.tensor.matmul(
                                ps[:, j:j + n],
                                lhsT,
                                col_sb[q:q + K, cstart + j:cstart + j + n],
                            )
                        nc.vector.tensor_reduce(
                            pm[:, i * n_ch + ci:i * n_ch + ci + 1], ps[:, :w],
                            axis=mybir.AxisListType.X, op=mybir.AluOpType.min,
                        )
                nc.vector.tensor_reduce(
                    rowmins[:], pm[:].rearrange("p (t c) -> p t c", c=n_ch),
                    axis=mybir.AxisListType.X, op=mybir.AluOpType.min,
                )
                nc.vector.reduce_sum(
                    out_sb[:, pi:pi + 1], rowmins[:], axis=mybir.AxisListType.X,
                )

            nc.sync.dma_start(out[:], out_sb[:])

    nc.compile()
    return nc


# --------------------------------------------------------------------------
# Host-side encode / shard / gather
# --------------------------------------------------------------------------

def _encode_rows(v: np.ndarray, c: int, P: int):
    """Row encoding [K,P] bf16 of the c real points in v (fake rows zero)."""
    row = np.zeros((K, P), np.float32)
    if c:
        v = v.astype(np.float32)
        vh = v.astype(BF16).astype(np.float32)
        vl = (v - vh).astype(BF16).astype(np.float32)
        n = (v.astype(np.float64) ** 2).sum(1)
        nh = n.astype(BF16).astype(np.float64)
        nl = (n - nh).astype(BF16).astype(np.float32)
        row[0:3, :c] = vh.T
        row[3:6, :c] = vl.T
        row[6:9, :c] = vh.T
        row[9, :c] = nh
        row[10, :c] = nl
        row[11, :c] = 1.0
        row[12, :c] = 1.0
    return row.astype(BF16)


def _encode_cols(v: np.ndarray, c: int, P: int, n_max: int):
    """Column encoding [K,P] bf16: c real candidate points, then (if the
    cloud is shorter than n_max) a zero-point candidate standing in for all
    to_dense_batch zero pads; alignment padding replicates a candidate."""
    q = np.zeros((P, 3), np.float32)
    w = np.ones(P, np.float32)
    n = np.zeros(P, np.float64)
    if c:
        v = v.astype(np.float32)
        q[:c] = v
        n[:c] = (v.astype(np.float64) ** 2).sum(1)
    if c and c >= n_max:
        # fullest graph: zero is NOT a candidate; pads replicate point 0
        q[c:] = v[0]
        n[c:] = n[0]
    # else: columns c.. stay the zero-point candidate (w=1, n=0)

    col = np.zeros((K, P), np.float32)
    m = (-2.0 * q).astype(np.float32)
    a = m.astype(BF16).astype(np.float32)
    b = (m - a).astype(BF16).astype(np.float32)
    nh = n.astype(BF16).astype(np.float64)
    nl = (n - nh).astype(BF16).astype(np.float32)
    col[0:3] = a.T
    col[3:6] = a.T
    col[6:9] = b.T
    col[9] = w
    col[10] = w
    col[11] = nh
    col[12] = nl
    return col.astype(BF16)


def prepare(pred, target, batch):
    """Returns (in_maps, num_graphs, n_max, P, gpc)."""
    pred = np.ascontiguousarray(np.asarray(pred), dtype=np.float32)
    target = np.ascontiguousarray(np.asarray(target), dtype=np.float32)
    batch = np.asarray(batch).astype(np.int64)

    num_graphs = int(batch.max()) + 1
    counts = np.bincount(batch, minlength=num_graphs)
    n_max = int(counts.max())
    P = ((n_max + 127) // 128) * 128
    gpc = max(1, math.ceil(num_graphs / N_CORES))
    starts = np.zeros(num_graphs + 1, np.int64)
    np.cumsum(counts, out=starts[1:])

    empty = np.zeros((0, 3), np.float32)
    in_maps = []
    for core in range(N_CORES):
        m = {k: np.zeros((gpc, K, P), BF16)
             for k in ("rowx", "colx", "rowy", "coly")}
        for slot in range(gpc):
            g = core * gpc + slot
            if g < num_graphs:
                c = int(counts[g])
                x = pred[starts[g]:starts[g + 1]]
                y = target[starts[g]:starts[g + 1]]
            else:
                c, x, y = 0, empty, empty  # unused slot contributes 0
            m["rowx"][slot] = _encode_rows(x, c, P)
            m["colx"][slot] = _encode_cols(x, c, P, n_max)
            m["rowy"][slot] = _encode_rows(y, c, P)
            m["coly"][slot] = _encode_cols(y, c, P, n_max)
        in_maps.append(m)
    return in_maps, num_graphs, n_max, P, gpc


def run(pred, target, batch, trace=False, **spmd_kwargs):
    """Full pipeline. Returns (loss_scalar, BassKernelResults)."""
    from concourse.bass_utils import run_bass_kernel_spmd

    in_maps, num_graphs, n_max, P, gpc = prepare(pred, target, batch)
    nc = build_nc(P, gpc)
    res = run_bass_kernel_spmd(
        nc, in_maps, core_ids=list(range(N_CORES)), trace=trace, **spmd_kwargs,
    )
    total = 0.0
    for core in range(N_CORES):
        total += res.results[core]["out"].astype(np.float64).sum()
    loss = np.float32(total / (num_graphs * n_max))
    return loss, res


def kernel(pred, target, batch):
    loss, _ = run(pred, target, batch, trace=False)
    return loss


# revision 26
# speedup vs baseline: 1.1670x; 1.1521x over previous
"""Chamfer-distance loss (nn_CDLoss) on 8 Trainium2 NeuronCores.

Strategy (data parallel over graphs, 2 graphs per core):

  Distances via one K=13 bf16 matmul (hi/lo split keeps fp32-grade accuracy;
  fp32 matmuls on TRN2 run in slow LOW_HIGH mode so we do the split ourselves,
  dropping only the lo*lo cross term, ~1e-6 relative on the loss):
      p = ph + pl (bf16 hi/lo), n_p = ||p||^2 = nh_p + nl_p (bf16 hi/lo)
      row-enc p : ( ph[3], pl[3], ph[3], nh_p, nl_p, 1, 1 )
      col-enc q : (-2qh[3], -2qh[3], -2ql[3], w, w, nh_q, nl_q )
      (row.T @ col)[p,q] = -2(ph qh + pl qh + ph ql) + w*n_p + n_q ~= ||p-q||^2

  Padding: fake rows are all-zero (their row-min is 0, adds nothing to the
  sum); to_dense_batch's zero pads are represented by one zero-point column;
  alignment pad columns replicate an existing candidate.

  Per (graph, direction): tile rows by 128 (PE row groups alternate q0/q32 so
  the next tile's weight load pulls ahead of in-flight matmuls), columns
  chunked 1024 wide through PSUM with 4 pool slots so the PE runs far enough
  ahead to keep the vector engine (the bottleneck) 100% busy on row-min
  reduces. Row-min partials land in pm, reduced to per-row mins, then per-lane
  sums are DMA'd out. Host sums the 8 cores' [128, 2*GPC] partials and
  divides by G*n_max.
"""

import math
import os
import sys

# concourse normally comes from PYTHONPATH (/root/.axon_site/_ro/trn_rl_repo);
# fall back to the /opt copy if the env var is missing.
for _p in ("/opt/trn_rl_repo", "/root/.axon_site/_ro/trn_rl_repo"):
    if os.path.isdir(_p) and _p not in sys.path:
        sys.path.append(_p)

import ml_dtypes
import numpy as np

BF16 = ml_dtypes.bfloat16
K = 13
N_CORES = 8


# --------------------------------------------------------------------------
# Device kernel
# --------------------------------------------------------------------------

def build_nc(P: int, gpc: int):
    """Build + compile the per-core Bass/Tile kernel.

    P   : padded points per cloud (multiple of 128)
    gpc : graphs per core
    Inputs  rowx, colx, rowy, coly : [gpc, K, P] bf16
    Output  out : [128, 2*gpc] f32 — per-lane sums of row-mins, one column
            per (graph, direction).
    """
    import concourse.bass as bass
    import concourse.mybir as mybir
    from concourse import bacc, tile

    f32 = mybir.dt.float32
    bf16 = mybir.dt.bfloat16
    T = P // 128
    # PSUM chunking of the column axis: 1024 f32 (2 banks) per chunk, 4 pool
    # slots = 8 banks. Tail chunk first so the post-row-tile-boundary
    # catch-up reduce is the small one.
    CH = 1024
    chunks = []
    c0 = 0
    while c0 < P:
        w = min(CH, P - c0)
        chunks.append((c0, w))
        c0 += w
    chunks.sort(key=lambda cw: cw[1])
    n_ch = len(chunks)

    nc = bacc.Bacc("TRN2", target_bir_lowering=False, debug=False)

    rowx = nc.dram_tensor("rowx", [gpc, K, P], bf16, kind="ExternalInput")
    colx = nc.dram_tensor("colx", [gpc, K, P], bf16, kind="ExternalInput")
    rowy = nc.dram_tensor("rowy", [gpc, K, P], bf16, kind="ExternalInput")
    coly = nc.dram_tensor("coly", [gpc, K, P], bf16, kind="ExternalInput")
    out = nc.dram_tensor("out", [128, 2 * gpc], f32, kind="ExternalOutput")

    # Big (1024-wide) chunks beyond the first are converted f32->bf16 by the
    # otherwise-idle scalar engine and row-min'd on the vector engine with
    # bf16 tensor_tensor halving (2x rate) + a short reduce; the first big
    # chunk and the tail stay on the direct fp32 PSUM reduce so ACT and DVE
    # finish a tile in about the same time.
    big_idx = [ci for ci, (_, w) in enumerate(chunks) if w == CH]
    conv_idx = set(big_idx[1:])
    n_conv = len(conv_idx)
    conv_w = n_conv * CH
    # pm columns per tile: direct chunks + one for the converted tree
    n_pm = (n_ch - n_conv) + (1 if n_conv else 0)

    with tile.TileContext(nc) as tc:
        with (
            tc.tile_pool(name="enc", bufs=2) as enc_pool,
            tc.tile_pool(name="conv", bufs=2) as conv_pool,
            tc.tile_pool(name="mins", bufs=2) as min_pool,
            tc.tile_pool(name="res", bufs=1) as res_pool,
            tc.tile_pool(name="ps", bufs=4, space="PSUM") as ps_pool,
        ):
            out_sb = res_pool.tile([128, 2 * gpc], f32, name="out_sb")

            pairs = []
            for g in range(gpc):
                pairs.append((rowx[g], coly[g]))  # cham_x direction
                pairs.append((rowy[g], colx[g]))  # cham_y direction

            for pi, (row_dram, col_dram) in enumerate(pairs):
                # Encodings replicated at partition offsets 0 and 32 so
                # consecutive row tiles use different PE row groups (q0/q32):
                # the next tile's LDWEIGHTS then pulls ahead of the in-flight
                # matmuls instead of waiting for the array to drain.
                row_sb = enc_pool.tile([32 + K, P], bf16, name="row_sb", tag="row")
                col_sb = enc_pool.tile([32 + K, P], bf16, name="col_sb", tag="col")
                nc.sync.dma_start(row_sb[0:K, :], row_dram)
                nc.sync.dma_start(row_sb[32:32 + K, :], row_dram)
                nc.sync.dma_start(col_sb[0:K, :], col_dram)
                nc.sync.dma_start(col_sb[32:32 + K, :], col_dram)

                # pm[:, i*n_pm + k] = row-min partials of row tile i
                pm = min_pool.tile([128, T * n_pm], f32, name="pm", tag="pm")
                rowmins = min_pool.tile([128, T], f32, name="rowmins", tag="rm")
                for i in range(T):
                    q = 32 * (i % 2)
                    lhsT = row_sb[q:q + K, i * 128:(i + 1) * 128]
                    conv = None
                    if n_conv:
                        conv = conv_pool.tile([128, conv_w], bf16,
                                              name="conv", tag="conv")
                    pmk = 0
                    nth_conv = 0
                    for ci, (cstart, w) in enumerate(chunks):
                        ps = ps_pool.tile([128, w], f32, name="ps", tag="ps")
                        for j in range(0, w, 512):
                            n = min(512, w - j)
                            nc.tensor.matmul(
                                ps[:, j:j + n],
                                lhsT,
                                col_sb[q:q + K, cstart + j:cstart + j + n],
                            )
                        if ci in conv_idx:
                            nc.scalar.copy(
                                conv[:, nth_conv * CH:(nth_conv + 1) * CH],
                                ps[:, :w],
                            )
                            nth_conv += 1
                        else:
                            nc.vector.tensor_reduce(
                                pm[:, i * n_pm + pmk:i * n_pm + pmk + 1],
                                ps[:, :w],
                                axis=mybir.AxisListType.X,
                                op=mybir.AluOpType.min,
                            )
                            pmk += 1
                    if n_conv:
                        # bf16 min-tree: halve in place at 2x until <=512 wide
                        hw = conv_w
                        while hw > 512:
                            hw //= 2
                            nc.vector.tensor_tensor(
                                conv[:, :hw], conv[:, :hw], conv[:, hw:2 * hw],
                                op=mybir.AluOpType.min,
                            )
                        nc.vector.tensor_reduce(
                            pm[:, i * n_pm + pmk:i * n_pm + pmk + 1],
                            conv[:, :hw],
                            axis=mybir.AxisListType.X, op=mybir.AluOpType.min,
                        )
                nc.vector.tensor_reduce(
                    rowmins[:], pm[:].rearrange("p (t c) -> p t c", c=n_pm),
                    axis=mybir.AxisListType.X, op=mybir.AluOpType.min,
                )
                nc.vector.reduce_sum(
                    out_sb[:, pi:pi + 1], rowmins[:], axis=mybir.AxisListType.X,
                )

            nc.sync.dma_start(out[:], out_sb[:])

    nc.compile()
    return nc


# --------------------------------------------------------------------------
# Host-side encode / shard / gather
# --------------------------------------------------------------------------

def _encode_rows(v: np.ndarray, c: int, P: int):
    """Row encoding [K,P] bf16 of the c real points in v (fake rows zero)."""
    row = np.zeros((K, P), np.float32)
    if c:
        v = v.astype(np.float32)
        vh = v.astype(BF16).astype(np.float32)
        vl = (v - vh).astype(BF16).astype(np.float32)
        n = (v.astype(np.float64) ** 2).sum(1)
        nh = n.astype(BF16).astype(np.float64)
        nl = (n - nh).astype(BF16).astype(np.float32)
        row[0:3, :c] = vh.T
        row[3:6, :c] = vl.T
        row[6:9, :c] = vh.T
        row[9, :c] = nh
        row[10, :c] = nl
        row[11, :c] = 1.0
        row[12, :c] = 1.0
    return row.astype(BF16)


def _encode_cols(v: np.ndarray, c: int, P: int, n_max: int):
    """Column encoding [K,P] bf16: c real candidate points, then (if the
    cloud is shorter than n_max) a zero-point candidate standing in for all
    to_dense_batch zero pads; alignment padding replicates a candidate."""
    q = np.zeros((P, 3), np.float32)
    w = np.ones(P, np.float32)
    n = np.zeros(P, np.float64)
    if c:
        v = v.astype(np.float32)
        q[:c] = v
        n[:c] = (v.astype(np.float64) ** 2).sum(1)
    if c and c >= n_max:
        # fullest graph: zero is NOT a candidate; pads replicate point 0
        q[c:] = v[0]
        n[c:] = n[0]
    # else: columns c.. stay the zero-point candidate (w=1, n=0)

    col = np.zeros((K, P), np.float32)
    m = (-2.0 * q).astype(np.float32)
    a = m.astype(BF16).astype(np.float32)
    b = (m - a).astype(BF16).astype(np.float32)
    nh = n.astype(BF16).astype(np.float64)
    nl = (n - nh).astype(BF16).astype(np.float32)
    col[0:3] = a.T
    col[3:6] = a.T
    col[6:9] = b.T
    col[9] = w
    col[10] = w
    col[11] = nh
    col[12] = nl
    return col.astype(BF16)


def prepare(pred, target, batch):
    """Returns (in_maps, num_graphs, n_max, P, gpc)."""
    pred = np.ascontiguousarray(np.asarray(pred), dtype=np.float32)
    target = np.ascontiguousarray(np.asarray(target), dtype=np.float32)
    batch = np.asarray(batch).astype(np.int64)

    num_graphs = int(batch.max()) + 1
    counts = np.bincount(batch, minlength=num_graphs)
    n_max = int(counts.max())
    P = ((n_max + 127) // 128) * 128
    gpc = max(1, math.ceil(num_graphs / N_CORES))
    starts = np.zeros(num_graphs + 1, np.int64)
    np.cumsum(counts, out=starts[1:])

    empty = np.zeros((0, 3), np.float32)
    in_maps = []
    for core in range(N_CORES):
        m = {k: np.zeros((gpc, K, P), BF16)
             for k in ("rowx", "colx", "rowy", "coly")}
        for slot in range(gpc):
            g = core * gpc + slot
            if g < num_graphs:
                c = int(counts[g])
                x = pred[starts[g]:starts[g + 1]]
                y = target[starts[g]:starts[g + 1]]
            else:
                c, x, y = 0, empty, empty  # unused slot contributes 0
            m["rowx"][slot] = _encode_rows(x, c, P)
            m["colx"][slot] = _encode_cols(x, c, P, n_max)
            m["rowy"][slot] = _encode_rows(y, c, P)
            m["coly"][slot] = _encode_cols(y, c, P, n_max)
        in_maps.append(m)
    return in_maps, num_graphs, n_max, P, gpc


def run(pred, target, batch, trace=False, **spmd_kwargs):
    """Full pipeline. Returns (loss_scalar, BassKernelResults)."""
    from concourse.bass_utils import run_bass_kernel_spmd

    in_maps, num_graphs, n_max, P, gpc = prepare(pred, target, batch)
    nc = build_nc(P, gpc)
    res = run_bass_kernel_spmd(
        nc, in_maps, core_ids=list(range(N_CORES)), trace=trace, **spmd_kwargs,
    )
    total = 0.0
    for core in range(N_CORES):
        total += res.results[core]["out"].astype(np.float64).sum()
    loss = np.float32(total / (num_graphs * n_max))
    return loss, res


def kernel(pred, target, batch):
    loss, _ = run(pred, target, batch, trace=False)
    return loss
